# revision 18
# baseline (speedup 1.0000x reference)
"""Edge-parallel COO SpMM on 8 trn2 cores — 4-queue SWDGE gather variant.

out[r] = sum_{e: rows[e]==r} vals[e] * seq[0, cols[e], :]

rows sorted -> core k owns 49 contiguous 128-row windows (6272 rows).
Per core:
  - tab[t] = [bf16(seq[2t]) | bf16(seq[2t+1])]  (256B tokens, 25000 rows);
    gather idx = col>>1 fits int16.
  - slots laid out window-major, each window padded only to the cross-core
    max count (NOT 128-aligned); chunk = 6-7 windows, padded to 128-multiple.
  - TWO dma_gather halves per chunk, round-robin over the 4 SWDGE queues.
    Q7 desc-gen (~8ns/idx) runs on a different Q7 cpu pair per queue and
    overlaps across queues (~3x measured) — this is the main speedup vs the
    single-queue variant.
  - DVE: one is_equal onehot S0 per half (vs iota 0..127); tiny S1 onehots
    (vs iota 128..255) for tiles where a window starts mid-tile; G scaled
    in-place by vev/vod (vals folded into G, not into two onehot copies).
  - PE per (window, tile): psum_w += S^T @ G_even + S^T @ G_odd.
  - flush psum -> stage -> one DMA to HBM.
Padding slots: idx=0 (valid token, no trailing-negative tricks), radj=-1000
(no onehot match), vev=vod=0 (zeroes G row) — NaN-safe double zero.
"""

import sys

if "/opt/trn_rl_repo" not in sys.path:
    sys.path.insert(0, "/opt/trn_rl_repo")

import numpy as np
import ml_dtypes

N_NODES = 50000
N_EDGES = 1_250_000
D_FEAT = 64
W = 128           # rows per window
NW = 49           # windows per core
NCORE = 8
NTOK = N_NODES // 2
NQUEUES = 4
CHUNK_SIZES = [4] * 11 + [3, 2]

_compiled = {}


def _make_plan(maxcnt):
    """Compile-time schedule from per-window cross-core max counts."""
    maxcnt = np.maximum(np.asarray(maxcnt, np.int64), 128)
    s = np.concatenate([[0], np.cumsum(maxcnt)])      # [NW+1] window offsets
    chunks = []
    tile_base = 0          # global tile index of chunk start
    w0 = 0
    for nw in CHUNK_SIZES:
        o = s[w0:w0 + nw + 1] - s[w0]                 # in-chunk window offsets
        nslots = int(o[-1])
        Cc = -(-nslots // 128)
        # per-window tile spans + variant (0: window starts at/before tile
        # start -> compare vs iota[0:128]; 1: window starts mid-tile ->
        # its radj is +128 biased -> compare vs iota[128:256])
        wplans = []
        btiles = set()
        for j in range(nw):
            t0 = int(o[j]) // 128
            t1 = -(-int(o[j + 1]) // 128)
            tiles = []
            for c in range(t0, t1):
                var = 1 if int(o[j]) > 128 * c else 0
                if var:
                    btiles.add(c)
                tiles.append((c, var, c == t0, c == t1 - 1))
            wplans.append(tiles)
        btiles = sorted(btiles)
        bpos = {c: i for i, c in enumerate(btiles)}
        Ch = (Cc + 1) // 2
        chunks.append(dict(
            w0=w0, nw=nw, Cc=Cc, Ch=Ch, tile_base=tile_base,
            wplans=wplans, btiles=btiles, bpos=bpos,
            slot_base=None,  # filled below
        ))
        tile_base += Cc
        w0 += nw
    T_TILES = tile_base
    # slot base of each chunk in the packed per-core arrays
    sb = 0
    for ch in chunks:
        ch["slot_base"] = sb
        sb += ch["Cc"] * 128
    return dict(maxcnt=maxcnt, s=s, chunks=chunks, T_TILES=T_TILES)


def _build_kernel(plan):
    from concourse import bass, bacc, mybir
    import concourse.tile as tile

    f32 = mybir.dt.float32
    bf16 = mybir.dt.bfloat16
    i16 = mybir.dt.int16

    T = plan["T_TILES"]

    nc = bacc.Bacc("TRN2", target_bir_lowering=False, debug=False,
                   num_swdge_queues=NQUEUES)
    tab_t = nc.dram_tensor("tab", [NTOK, 128], bf16, kind="ExternalInput")
    idx_t = nc.dram_tensor("idx", [128, T * 8], i16, kind="ExternalInput")
    radj_t = nc.dram_tensor("radj", [128, T], bf16, kind="ExternalInput")
    vev_t = nc.dram_tensor("vev", [128, T], f32, kind="ExternalInput")
    vod_t = nc.dram_tensor("vod", [128, T], f32, kind="ExternalInput")
    out_t = nc.dram_tensor("out", [128, NW * 64], f32, kind="ExternalOutput")

    # greedy queue assignment: each half-gather to the least-loaded queue
    qload = [0] * NQUEUES

    def pick_queue(n):
        q = min(range(NQUEUES), key=lambda i: qload[i])
        qload[q] += n
        return q

    with tile.TileContext(nc) as tc:
        with (
            tc.tile_pool(name="const", bufs=1) as constp,
            tc.tile_pool(name="g", bufs=4) as gp,
            tc.tile_pool(name="s", bufs=1) as sp,
            tc.tile_pool(name="ps", bufs=4, space="PSUM") as psp,
            tc.tile_pool(name="st", bufs=1) as stp,
        ):
            iota_t = constp.tile([128, 256], bf16, name="iota")
            nc.gpsimd.iota(
                iota_t[:],
                pattern=[[1, 256]],
                base=0,
                channel_multiplier=0,
                allow_small_or_imprecise_dtypes=True,
            )
            # all metadata upfront: no per-chunk meta dependency chains
            idxa = constp.tile([128, T * 8], i16, name="idxa")
            rada = constp.tile([128, T], bf16, name="rada")
            veva = constp.tile([128, T], f32, name="veva")
            voda = constp.tile([128, T], f32, name="voda")
            nc.sync.dma_start(out=idxa[:], in_=idx_t[:, :])
            nc.sync.dma_start(out=rada[:], in_=radj_t[:, :])
            nc.sync.dma_start(out=veva[:], in_=vev_t[:, :])
            nc.sync.dma_start(out=voda[:], in_=vod_t[:, :])
            stage = stp.tile([128, NW * 64], f32, name="stage")

            for ch in plan["chunks"]:
                Cc, Ch = ch["Cc"], ch["Ch"]
                tb = ch["tile_base"]
                w0, nw = ch["w0"], ch["nw"]

                G = gp.tile([128, Cc, 128], bf16, tag="g")
                for h0, h1 in ((0, Ch), (Ch, Cc)):
                    nc.gpsimd.dma_gather(
                        G[:, h0:h1, :], tab_t[:, :],
                        idxa[:, (tb + h0) * 8:(tb + h1) * 8],
                        (h1 - h0) * 128, (h1 - h0) * 128, 128,
                        single_packet=False,
                        queue_num=pick_queue(h1 - h0),
                    )

                ps = psp.tile([128, 512], f32, tag="ps")
                S1l = {}
                for h0, h1 in ((0, Ch), (Ch, Cc)):
                    hc = h1 - h0
                    S0 = sp.tile([128, hc, 128], bf16, tag=f"s0{h0 > 0}")
                    nc.vector.tensor_tensor(
                        out=S0[:, :, :],
                        in0=iota_t[:, None, 0:128].to_broadcast([128, hc, 128]),
                        in1=rada[:, tb + h0:tb + h1].to_broadcast([128, hc, 128]),
                        op=mybir.AluOpType.is_equal,
                    )
                    bt_h = [c for c in ch["btiles"] if h0 <= c < h1]
                    if bt_h:
                        S1 = sp.tile([128, len(bt_h), 128], bf16,
                                     tag=f"s1{h0 > 0}")
                        S1l[h0 > 0] = (S1, {c: i for i, c in enumerate(bt_h)})
                        for i, c in enumerate(bt_h):
                            nc.vector.tensor_tensor(
                                out=S1[:, i:i + 1, :],
                                in0=iota_t[:, None, 128:256].to_broadcast(
                                    [128, 1, 128]),
                                in1=rada[:, tb + c:tb + c + 1].to_broadcast(
                                    [128, 1, 128]),
                                op=mybir.AluOpType.is_equal,
                            )
                    # fold vals into G on the (otherwise idle) Act engine so
                    # the in-order DVE never stalls waiting for gather drains
                    for c in range(h0, h1):
                        nc.scalar.activation(
                            out=G[:, c, 0:64], in_=G[:, c, 0:64],
                            func=mybir.ActivationFunctionType.Copy,
                            scale=veva[:, tb + c:tb + c + 1],
                        )
                        nc.scalar.activation(
                            out=G[:, c, 64:128], in_=G[:, c, 64:128],
                            func=mybir.ActivationFunctionType.Copy,
                            scale=voda[:, tb + c:tb + c + 1],
                        )
                    for j in range(nw):
                        for (c, var, first, last) in ch["wplans"][j]:
                            if not (h0 <= c < h1):
                                continue
                            if var:
                                S1, pmap = S1l[h0 > 0]
                                lhsT = S1[:, pmap[c], :]
                            else:
                                lhsT = S0[:, c - h0, :]
                            nc.tensor.matmul(
                                out=ps[:, j * 64:(j + 1) * 64],
                                lhsT=lhsT,
                                rhs=G[:, c, 0:64],
                                start=first,
                                stop=False,
                            )
                            nc.tensor.matmul(
                                out=ps[:, j * 64:(j + 1) * 64],
                                lhsT=lhsT,
                                rhs=G[:, c, 64:128],
                                start=False,
                                stop=last,
                            )

                nc.scalar.activation(
                    out=stage[:, w0 * 64:(w0 + nw) * 64],
                    in_=ps[:, :nw * 64],
                    func=mybir.ActivationFunctionType.Copy,
                )
                nc.sync.dma_start(
                    out=out_t[:, w0 * 64:(w0 + nw) * 64],
                    in_=stage[:, w0 * 64:(w0 + nw) * 64],
                )

    nc.compile()
    return nc


def _preprocess(seq, vals, rows, cols):
    rows = np.asarray(rows)
    cols = np.asarray(cols)
    vals = np.asarray(vals)

    w_of_edge = rows // W
    counts = np.bincount(w_of_edge, minlength=NCORE * NW)
    starts = np.concatenate([[0], np.cumsum(counts)])
    cnt = counts.reshape(NCORE, NW)
    maxcnt = np.maximum(cnt.max(axis=0), 128)

    plan = _make_plan(maxcnt)
    chunks = plan["chunks"]
    T = plan["T_TILES"]
    TS = T * 128

    # window -> (chunk, in-chunk offset) slot position
    wslot = np.zeros(NW, np.int64)
    s = plan["s"]
    for ch in chunks:
        for j in range(ch["nw"]):
            w = ch["w0"] + j
            wslot[w] = ch["slot_base"] + (s[w] - s[ch["w0"]])

    # per-slot tile-first-window bias: radj = row - (base window)*128 where
    # base window = window owning the first slot of the slot's tile.
    # For a window starting mid-tile, its first slots get +128 bias.
    idx16 = np.zeros((NCORE, TS), np.int16)
    radj = np.full((NCORE, TS), -1000.0, np.float32)
    vev = np.zeros((NCORE, TS), np.float32)
    vod = np.zeros((NCORE, TS), np.float32)

    for k in range(NCORE):
        for i in range(NW):
            g = k * NW + i
            a, b = int(starts[g]), int(starts[g + 1])
            n = b - a
            if n == 0:
                continue
            p0 = int(wslot[i])                      # first slot of window i
            sl = np.arange(p0, p0 + n)
            c = cols[a:b]
            idx16[k, sl] = (c >> 1).astype(np.int16)
            # variant bias: slots whose tile starts before the window start
            tile_of = sl // 128
            first_tile_start = tile_of * 128
            bias = np.where(first_tile_start < p0, 128, 0)
            # only the window's FIRST tile can start before p0
            radj[k, sl] = (rows[a:b] - g * W).astype(np.float32) + bias
            par = (c & 1).astype(np.float32)
            v = vals[a:b]
            vev[k, sl] = v * (1.0 - par)
            vod[k, sl] = v * par

    def wrap16(a):
        t = a.reshape(-1, 16).T
        return np.ascontiguousarray(np.tile(t, (8, 1)))

    seqb = np.asarray(seq).reshape(N_NODES, D_FEAT).astype(ml_dtypes.bfloat16)
    tab = np.ascontiguousarray(seqb.reshape(NTOK, 128))

    bf = ml_dtypes.bfloat16
    in_maps = []
    for k in range(NCORE):
        in_maps.append(
            {
                "tab": tab,
                "idx": wrap16(idx16[k]),
                "radj": np.ascontiguousarray(radj[k].reshape(T, 128).T).astype(bf),
                "vev": np.ascontiguousarray(vev[k].reshape(T, 128).T),
                "vod": np.ascontiguousarray(vod[k].reshape(T, 128).T),
            }
        )
    return plan, in_maps


def kernel(seq, vals, rows, cols, _trace=False):
    from concourse.bass_utils import run_bass_kernel_spmd

    plan, in_maps = _preprocess(seq, vals, rows, cols)

    key = tuple(int(x) for x in plan["maxcnt"])
    if key not in _compiled:
        _compiled[key] = _build_kernel(plan)
    nc = _compiled[key]

    res = run_bass_kernel_spmd(nc, in_maps, core_ids=list(range(NCORE)), trace=_trace)

    outs = []
    for k in range(NCORE):
        o = res.results[k]["out"]                        # [128, 3136]
        outs.append(o.reshape(128, NW, 64).transpose(1, 0, 2).reshape(NW * W, 64))
    full = np.concatenate(outs, axis=0)[:N_NODES]
    out = full[None].astype(np.float32)
    if _trace:
        return out, res
    return out


# revision 22
# speedup vs baseline: 1.2556x; 1.2556x over previous
"""Edge-parallel COO SpMM on 8 trn2 cores — 4-queue SWDGE gather variant.

out[r] = sum_{e: rows[e]==r} vals[e] * seq[0, cols[e], :]

rows sorted -> core k owns 49 contiguous 128-row windows (6272 rows).
Per core:
  - tab[t] = [bf16(seq[2t]) | bf16(seq[2t+1])]  (256B tokens, 25000 rows);
    gather idx = col>>1 fits int16.
  - slots laid out window-major, each window padded only to the cross-core
    max count (NOT 128-aligned); chunk = 6-7 windows, padded to 128-multiple.
  - TWO dma_gather halves per chunk, round-robin over the 4 SWDGE queues.
    Q7 desc-gen (~8ns/idx) runs on a different Q7 cpu pair per queue and
    overlaps across queues (~3x measured) — this is the main speedup vs the
    single-queue variant.
  - DVE: one is_equal onehot S0 per half (vs iota 0..127); tiny S1 onehots
    (vs iota 128..255) for tiles where a window starts mid-tile; G scaled
    in-place by vev/vod (vals folded into G, not into two onehot copies).
  - PE per (window, tile): psum_w += S^T @ G_even + S^T @ G_odd.
  - flush psum -> stage -> one DMA to HBM.
Padding slots: idx=0 (valid token, no trailing-negative tricks), radj=-1000
(no onehot match), vev=vod=0 (zeroes G row) — NaN-safe double zero.
"""

import sys

if "/opt/trn_rl_repo" not in sys.path:
    sys.path.insert(0, "/opt/trn_rl_repo")

import numpy as np
import ml_dtypes

N_NODES = 50000
N_EDGES = 1_250_000
D_FEAT = 64
W = 128           # rows per window
NW = 49           # windows per core
NCORE = 8
NTOK = N_NODES // 2
NQUEUES = 4
CHUNK_SIZES = [4] * 11 + [3, 2]

_compiled = {}


def _make_plan(maxcnt):
    """Compile-time schedule from per-window cross-core max counts."""
    maxcnt = np.maximum(np.asarray(maxcnt, np.int64), 128)
    s = np.concatenate([[0], np.cumsum(maxcnt)])      # [NW+1] window offsets
    chunks = []
    tile_base = 0          # global tile index of chunk start
    w0 = 0
    for nw in CHUNK_SIZES:
        o = s[w0:w0 + nw + 1] - s[w0]                 # in-chunk window offsets
        nslots = int(o[-1])
        Cc = -(-nslots // 128)
        # per-window tile spans + variant (0: window starts at/before tile
        # start -> compare vs iota[0:128]; 1: window starts mid-tile ->
        # its radj is +128 biased -> compare vs iota[128:256])
        wplans = []
        btiles = set()
        for j in range(nw):
            t0 = int(o[j]) // 128
            t1 = -(-int(o[j + 1]) // 128)
            tiles = []
            for c in range(t0, t1):
                var = 1 if int(o[j]) > 128 * c else 0
                if var:
                    btiles.add(c)
                tiles.append((c, var, c == t0, c == t1 - 1))
            wplans.append(tiles)
        btiles = sorted(btiles)
        bpos = {c: i for i, c in enumerate(btiles)}
        Ch = (Cc + 1) // 2
        chunks.append(dict(
            w0=w0, nw=nw, Cc=Cc, Ch=Ch, tile_base=tile_base,
            wplans=wplans, btiles=btiles, bpos=bpos,
            slot_base=None,  # filled below
        ))
        tile_base += Cc
        w0 += nw
    T_TILES = tile_base
    # slot base of each chunk in the packed per-core arrays
    sb = 0
    for ch in chunks:
        ch["slot_base"] = sb
        sb += ch["Cc"] * 128
    return dict(maxcnt=maxcnt, s=s, chunks=chunks, T_TILES=T_TILES)


def _build_kernel(plan):
    from concourse import bass, bacc, mybir
    import concourse.tile as tile

    f32 = mybir.dt.float32
    bf16 = mybir.dt.bfloat16
    i16 = mybir.dt.int16

    T = plan["T_TILES"]

    nc = bacc.Bacc("TRN2", target_bir_lowering=False, debug=False,
                   num_swdge_queues=NQUEUES)
    tab_t = nc.dram_tensor("tab", [NTOK, 128], bf16, kind="ExternalInput")
    idx_t = nc.dram_tensor("idx", [128, T * 8], i16, kind="ExternalInput")
    radj_t = nc.dram_tensor("radj", [128, T], bf16, kind="ExternalInput")
    vev_t = nc.dram_tensor("vev", [128, T], bf16, kind="ExternalInput")
    vod_t = nc.dram_tensor("vod", [128, T], bf16, kind="ExternalInput")
    out_t = nc.dram_tensor("out", [128, NW * 64], f32, kind="ExternalOutput")

    # greedy queue assignment: each half-gather to the least-loaded queue
    qload = [0] * NQUEUES

    def pick_queue(n):
        q = min(range(NQUEUES), key=lambda i: qload[i])
        qload[q] += n
        return q

    with tile.TileContext(nc) as tc:
        with (
            tc.tile_pool(name="const", bufs=1) as constp,
            tc.tile_pool(name="g", bufs=4) as gp,
            tc.tile_pool(name="s", bufs=1) as sp,
            tc.tile_pool(name="ps", bufs=4, space="PSUM") as psp,
            tc.tile_pool(name="st", bufs=1) as stp,
        ):
            iota_t = constp.tile([128, 256], bf16, name="iota")
            nc.gpsimd.iota(
                iota_t[:],
                pattern=[[1, 256]],
                base=0,
                channel_multiplier=0,
                allow_small_or_imprecise_dtypes=True,
            )
            # all metadata upfront: no per-chunk meta dependency chains
            idxa = constp.tile([128, T * 8], i16, name="idxa")
            rada = constp.tile([128, T], bf16, name="rada")
            veva = constp.tile([128, T], bf16, name="veva")
            voda = constp.tile([128, T], bf16, name="voda")
            nc.sync.dma_start(out=idxa[:], in_=idx_t[:, :])
            nc.sync.dma_start(out=rada[:], in_=radj_t[:, :])
            nc.sync.dma_start(out=veva[:], in_=vev_t[:, :])
            nc.sync.dma_start(out=voda[:], in_=vod_t[:, :])
            stage = stp.tile([128, NW * 64], f32, name="stage")

            for ch in plan["chunks"]:
                Cc, Ch = ch["Cc"], ch["Ch"]
                tb = ch["tile_base"]
                w0, nw = ch["w0"], ch["nw"]

                G = gp.tile([128, Cc, 128], bf16, tag="g")
                for h0, h1 in ((0, Ch), (Ch, Cc)):
                    nc.gpsimd.dma_gather(
                        G[:, h0:h1, :], tab_t[:, :],
                        idxa[:, (tb + h0) * 8:(tb + h1) * 8],
                        (h1 - h0) * 128, (h1 - h0) * 128, 128,
                        single_packet=False,
                        queue_num=pick_queue(h1 - h0),
                    )

                ps = psp.tile([128, 512], f32, tag="ps")
                S1l = {}
                for h0, h1 in ((0, Ch), (Ch, Cc)):
                    hc = h1 - h0
                    S0 = sp.tile([128, hc, 128], bf16, tag=f"s0{h0 > 0}")
                    nc.vector.tensor_tensor(
                        out=S0[:, :, :],
                        in0=iota_t[:, None, 0:128].to_broadcast([128, hc, 128]),
                        in1=rada[:, tb + h0:tb + h1].to_broadcast([128, hc, 128]),
                        op=mybir.AluOpType.is_equal,
                    )
                    bt_h = [c for c in ch["btiles"] if h0 <= c < h1]
                    if bt_h:
                        S1 = sp.tile([128, len(bt_h), 128], bf16,
                                     tag=f"s1{h0 > 0}")
                        S1l[h0 > 0] = (S1, {c: i for i, c in enumerate(bt_h)})
                        for i, c in enumerate(bt_h):
                            nc.vector.tensor_tensor(
                                out=S1[:, i:i + 1, :],
                                in0=iota_t[:, None, 128:256].to_broadcast(
                                    [128, 1, 128]),
                                in1=rada[:, tb + c:tb + c + 1].to_broadcast(
                                    [128, 1, 128]),
                                op=mybir.AluOpType.is_equal,
                            )
                    # fold vals into G (even nodes *= vev, odd *= vod)
                    nc.vector.tensor_tensor(
                        out=G[:, h0:h1, 0:64],
                        in0=G[:, h0:h1, 0:64],
                        in1=veva[:, tb + h0:tb + h1].to_broadcast([128, hc, 64]),
                        op=mybir.AluOpType.mult,
                    )
                    nc.vector.tensor_tensor(
                        out=G[:, h0:h1, 64:128],
                        in0=G[:, h0:h1, 64:128],
                        in1=voda[:, tb + h0:tb + h1].to_broadcast([128, hc, 64]),
                        op=mybir.AluOpType.mult,
                    )
                    for j in range(nw):
                        for (c, var, first, last) in ch["wplans"][j]:
                            if not (h0 <= c < h1):
                                continue
                            if var:
                                S1, pmap = S1l[h0 > 0]
                                lhsT = S1[:, pmap[c], :]
                            else:
                                lhsT = S0[:, c - h0, :]
                            nc.tensor.matmul(
                                out=ps[:, j * 64:(j + 1) * 64],
                                lhsT=lhsT,
                                rhs=G[:, c, 0:64],
                                start=first,
                                stop=False,
                            )
                            nc.tensor.matmul(
                                out=ps[:, j * 64:(j + 1) * 64],
                                lhsT=lhsT,
                                rhs=G[:, c, 64:128],
                                start=False,
                                stop=last,
                            )

                nc.scalar.activation(
                    out=stage[:, w0 * 64:(w0 + nw) * 64],
                    in_=ps[:, :nw * 64],
                    func=mybir.ActivationFunctionType.Copy,
                )
                nc.sync.dma_start(
                    out=out_t[:, w0 * 64:(w0 + nw) * 64],
                    in_=stage[:, w0 * 64:(w0 + nw) * 64],
                )

    nc.compile()
    return nc


def _preprocess(seq, vals, rows, cols):
    rows = np.asarray(rows)
    cols = np.asarray(cols)
    vals = np.asarray(vals)

    w_of_edge = rows // W
    counts = np.bincount(w_of_edge, minlength=NCORE * NW)
    starts = np.concatenate([[0], np.cumsum(counts)])
    cnt = counts.reshape(NCORE, NW)
    maxcnt = np.maximum(cnt.max(axis=0), 128)

    plan = _make_plan(maxcnt)
    chunks = plan["chunks"]
    T = plan["T_TILES"]
    TS = T * 128

    # window -> (chunk, in-chunk offset) slot position
    wslot = np.zeros(NW, np.int64)
    s = plan["s"]
    for ch in chunks:
        for j in range(ch["nw"]):
            w = ch["w0"] + j
            wslot[w] = ch["slot_base"] + (s[w] - s[ch["w0"]])

    # per-slot tile-first-window bias: radj = row - (base window)*128 where
    # base window = window owning the first slot of the slot's tile.
    # For a window starting mid-tile, its first slots get +128 bias.
    idx16 = np.zeros((NCORE, TS), np.int16)
    radj = np.full((NCORE, TS), -1000.0, np.float32)
    vev = np.zeros((NCORE, TS), np.float32)
    vod = np.zeros((NCORE, TS), np.float32)

    for k in range(NCORE):
        for i in range(NW):
            g = k * NW + i
            a, b = int(starts[g]), int(starts[g + 1])
            n = b - a
            if n == 0:
                continue
            p0 = int(wslot[i])                      # first slot of window i
            sl = np.arange(p0, p0 + n)
            c = cols[a:b]
            idx16[k, sl] = (c >> 1).astype(np.int16)
            # variant bias: slots whose tile starts before the window start
            tile_of = sl // 128
            first_tile_start = tile_of * 128
            bias = np.where(first_tile_start < p0, 128, 0)
            # only the window's FIRST tile can start before p0
            radj[k, sl] = (rows[a:b] - g * W).astype(np.float32) + bias
            par = (c & 1).astype(np.float32)
            v = vals[a:b]
            vev[k, sl] = v * (1.0 - par)
            vod[k, sl] = v * par

    def wrap16(a):
        t = a.reshape(-1, 16).T
        return np.ascontiguousarray(np.tile(t, (8, 1)))

    seqb = np.asarray(seq).reshape(N_NODES, D_FEAT).astype(ml_dtypes.bfloat16)
    tab = np.ascontiguousarray(seqb.reshape(NTOK, 128))

    bf = ml_dtypes.bfloat16
    in_maps = []
    for k in range(NCORE):
        in_maps.append(
            {
                "tab": tab,
                "idx": wrap16(idx16[k]),
                "radj": np.ascontiguousarray(radj[k].reshape(T, 128).T).astype(bf),
                "vev": np.ascontiguousarray(vev[k].reshape(T, 128).T).astype(bf),
                "vod": np.ascontiguousarray(vod[k].reshape(T, 128).T).astype(bf),
            }
        )
    return plan, in_maps


def kernel(seq, vals, rows, cols, _trace=False):
    from concourse.bass_utils import run_bass_kernel_spmd

    plan, in_maps = _preprocess(seq, vals, rows, cols)

    key = tuple(int(x) for x in plan["maxcnt"])
    if key not in _compiled:
        _compiled[key] = _build_kernel(plan)
    nc = _compiled[key]

    res = run_bass_kernel_spmd(nc, in_maps, core_ids=list(range(NCORE)), trace=_trace)

    outs = []
    for k in range(NCORE):
        o = res.results[k]["out"]                        # [128, 3136]
        outs.append(o.reshape(128, NW, 64).transpose(1, 0, 2).reshape(NW * W, 64))
    full = np.concatenate(outs, axis=0)[:N_NODES]
    out = full[None].astype(np.float32)
    if _trace:
        return out, res
    return out


# revision 24
# speedup vs baseline: 1.7793x; 1.4172x over previous
"""Edge-parallel COO SpMM on 8 trn2 cores — 4-queue SWDGE gather variant.

out[r] = sum_{e: rows[e]==r} vals[e] * seq[0, cols[e], :]

rows sorted -> core k owns 49 contiguous 128-row windows (6272 rows).
Per core:
  - tab[t] = [bf16(seq[2t]) | bf16(seq[2t+1])]  (256B tokens, 25000 rows);
    gather idx = col>>1 fits int16.
  - slots laid out window-major, each window padded only to the cross-core
    max count (NOT 128-aligned); chunk = 6-7 windows, padded to 128-multiple.
  - TWO dma_gather halves per chunk, round-robin over the 4 SWDGE queues.
    Q7 desc-gen (~8ns/idx) runs on a different Q7 cpu pair per queue and
    overlaps across queues (~3x measured) — this is the main speedup vs the
    single-queue variant.
  - DVE: one is_equal onehot S0 per half (vs iota 0..127); tiny S1 onehots
    (vs iota 128..255) for tiles where a window starts mid-tile; G scaled
    in-place by vev/vod (vals folded into G, not into two onehot copies).
  - PE per (window, tile): psum_w += S^T @ G_even + S^T @ G_odd.
  - flush psum -> stage -> one DMA to HBM.
Padding slots: idx=0 (valid token, no trailing-negative tricks), radj=-1000
(no onehot match), vev=vod=0 (zeroes G row) — NaN-safe double zero.
"""

import sys

if "/opt/trn_rl_repo" not in sys.path:
    sys.path.insert(0, "/opt/trn_rl_repo")

import numpy as np
import ml_dtypes

N_NODES = 50000
N_EDGES = 1_250_000
D_FEAT = 64
W = 128           # rows per window
NW = 49           # windows per core
NCORE = 8
NTOK = N_NODES // 2
NQUEUES = 4
CHUNK_SIZES = [5] + [4] * 11

_compiled = {}


def _make_plan(maxcnt):
    """Compile-time schedule from per-window cross-core max counts."""
    maxcnt = np.maximum(np.asarray(maxcnt, np.int64), 128)
    s = np.concatenate([[0], np.cumsum(maxcnt)])      # [NW+1] window offsets
    chunks = []
    tile_base = 0          # global tile index of chunk start
    w0 = 0
    for nw in CHUNK_SIZES:
        o = s[w0:w0 + nw + 1] - s[w0]                 # in-chunk window offsets
        nslots = int(o[-1])
        Cc = -(-nslots // 128)
        # per-window tile spans + variant (0: window starts at/before tile
        # start -> compare vs iota[0:128]; 1: window starts mid-tile ->
        # its radj is +128 biased -> compare vs iota[128:256])
        wplans = []
        btiles = set()
        for j in range(nw):
            t0 = int(o[j]) // 128
            t1 = -(-int(o[j + 1]) // 128)
            tiles = []
            for c in range(t0, t1):
                var = 1 if int(o[j]) > 128 * c else 0
                if var:
                    btiles.add(c)
                tiles.append((c, var, c == t0, c == t1 - 1))
            wplans.append(tiles)
        btiles = sorted(btiles)
        bpos = {c: i for i, c in enumerate(btiles)}
        Ch = (Cc + 1) // 2
        chunks.append(dict(
            w0=w0, nw=nw, Cc=Cc, Ch=Ch, tile_base=tile_base,
            wplans=wplans, btiles=btiles, bpos=bpos,
            slot_base=None,  # filled below
        ))
        tile_base += Cc
        w0 += nw
    T_TILES = tile_base
    # slot base of each chunk in the packed per-core arrays
    sb = 0
    for ch in chunks:
        ch["slot_base"] = sb
        sb += ch["Cc"] * 128
    return dict(maxcnt=maxcnt, s=s, chunks=chunks, T_TILES=T_TILES)


def _build_kernel(plan):
    from concourse import bass, bacc, mybir
    import concourse.tile as tile

    f32 = mybir.dt.float32
    bf16 = mybir.dt.bfloat16
    i16 = mybir.dt.int16

    T = plan["T_TILES"]

    nc = bacc.Bacc("TRN2", target_bir_lowering=False, debug=False,
                   num_swdge_queues=NQUEUES)
    tab_t = nc.dram_tensor("tab", [NTOK, 128], bf16, kind="ExternalInput")
    idx_t = nc.dram_tensor("idx", [128, T * 8], i16, kind="ExternalInput")
    radj_t = nc.dram_tensor("radj", [128, T], bf16, kind="ExternalInput")
    vev_t = nc.dram_tensor("vev", [128, T], bf16, kind="ExternalInput")
    vod_t = nc.dram_tensor("vod", [128, T], bf16, kind="ExternalInput")
    out_t = nc.dram_tensor("out", [128, NW * 64], f32, kind="ExternalOutput")

    # round-robin queue assignment
    qctr = [0]

    def pick_queue(n):
        q = qctr[0] % NQUEUES
        qctr[0] += 1
        return q

    with tile.TileContext(nc) as tc:
        with (
            tc.tile_pool(name="const", bufs=1) as constp,
            tc.tile_pool(name="g", bufs=4) as gp,
            tc.tile_pool(name="s", bufs=1) as sp,
            tc.tile_pool(name="ps", bufs=4, space="PSUM") as psp,
            tc.tile_pool(name="st", bufs=1) as stp,
        ):
            iota_t = constp.tile([128, 256], bf16, name="iota")
            nc.gpsimd.iota(
                iota_t[:],
                pattern=[[1, 256]],
                base=0,
                channel_multiplier=0,
                allow_small_or_imprecise_dtypes=True,
            )
            # all metadata upfront: no per-chunk meta dependency chains
            idxa = constp.tile([128, T * 8], i16, name="idxa")
            rada = constp.tile([128, T], bf16, name="rada")
            veva = constp.tile([128, T], bf16, name="veva")
            voda = constp.tile([128, T], bf16, name="voda")
            nc.sync.dma_start(out=idxa[:], in_=idx_t[:, :])
            nc.sync.dma_start(out=rada[:], in_=radj_t[:, :])
            nc.sync.dma_start(out=veva[:], in_=vev_t[:, :])
            nc.sync.dma_start(out=voda[:], in_=vod_t[:, :])
            stage = stp.tile([128, NW * 64], f32, name="stage")

            for ch in plan["chunks"]:
                Cc, Ch = ch["Cc"], ch["Ch"]
                tb = ch["tile_base"]
                w0, nw = ch["w0"], ch["nw"]

                G = gp.tile([128, Cc, 128], bf16, tag="g")
                for h0, h1 in ((0, Ch), (Ch, Cc)):
                    nc.gpsimd.dma_gather(
                        G[:, h0:h1, :], tab_t[:, :],
                        idxa[:, (tb + h0) * 8:(tb + h1) * 8],
                        (h1 - h0) * 128, (h1 - h0) * 128, 128,
                        single_packet=False,
                        queue_num=pick_queue(h1 - h0),
                    )

                ps = psp.tile([128, 512], f32, tag="ps")
                S1l = {}
                for h0, h1 in ((0, Ch), (Ch, Cc)):
                    hc = h1 - h0
                    S0 = sp.tile([128, hc, 128], bf16, tag=f"s0{h0 > 0}")
                    nc.vector.tensor_tensor(
                        out=S0[:, :, :],
                        in0=iota_t[:, None, 0:128].to_broadcast([128, hc, 128]),
                        in1=rada[:, tb + h0:tb + h1].to_broadcast([128, hc, 128]),
                        op=mybir.AluOpType.is_equal,
                    )
                    bt_h = [c for c in ch["btiles"] if h0 <= c < h1]
                    if bt_h:
                        S1 = sp.tile([128, len(bt_h), 128], bf16,
                                     tag=f"s1{h0 > 0}")
                        S1l[h0 > 0] = (S1, {c: i for i, c in enumerate(bt_h)})
                        for i, c in enumerate(bt_h):
                            nc.vector.tensor_tensor(
                                out=S1[:, i:i + 1, :],
                                in0=iota_t[:, None, 128:256].to_broadcast(
                                    [128, 1, 128]),
                                in1=rada[:, tb + c:tb + c + 1].to_broadcast(
                                    [128, 1, 128]),
                                op=mybir.AluOpType.is_equal,
                            )
                    # fold vals into G (even nodes *= vev, odd *= vod)
                    nc.vector.tensor_tensor(
                        out=G[:, h0:h1, 0:64],
                        in0=G[:, h0:h1, 0:64],
                        in1=veva[:, tb + h0:tb + h1].to_broadcast([128, hc, 64]),
                        op=mybir.AluOpType.mult,
                    )
                    nc.vector.tensor_tensor(
                        out=G[:, h0:h1, 64:128],
                        in0=G[:, h0:h1, 64:128],
                        in1=voda[:, tb + h0:tb + h1].to_broadcast([128, hc, 64]),
                        op=mybir.AluOpType.mult,
                    )
                    for j in range(nw):
                        for (c, var, first, last) in ch["wplans"][j]:
                            if not (h0 <= c < h1):
                                continue
                            if var:
                                S1, pmap = S1l[h0 > 0]
                                lhsT = S1[:, pmap[c], :]
                            else:
                                lhsT = S0[:, c - h0, :]
                            nc.tensor.matmul(
                                out=ps[:, j * 64:(j + 1) * 64],
                                lhsT=lhsT,
                                rhs=G[:, c, 0:64],
                                start=first,
                                stop=False,
                            )
                            nc.tensor.matmul(
                                out=ps[:, j * 64:(j + 1) * 64],
                                lhsT=lhsT,
                                rhs=G[:, c, 64:128],
                                start=False,
                                stop=last,
                            )

                nc.scalar.activation(
                    out=stage[:, w0 * 64:(w0 + nw) * 64],
                    in_=ps[:, :nw * 64],
                    func=mybir.ActivationFunctionType.Copy,
                )
                nc.sync.dma_start(
                    out=out_t[:, w0 * 64:(w0 + nw) * 64],
                    in_=stage[:, w0 * 64:(w0 + nw) * 64],
                )

    nc.compile()
    return nc


def _preprocess(seq, vals, rows, cols):
    rows = np.asarray(rows)
    cols = np.asarray(cols)
    vals = np.asarray(vals)

    w_of_edge = rows // W
    counts = np.bincount(w_of_edge, minlength=NCORE * NW)
    starts = np.concatenate([[0], np.cumsum(counts)])
    cnt = counts.reshape(NCORE, NW)
    maxcnt = np.maximum(cnt.max(axis=0), 128)

    plan = _make_plan(maxcnt)
    chunks = plan["chunks"]
    T = plan["T_TILES"]
    TS = T * 128

    # window -> (chunk, in-chunk offset) slot position
    wslot = np.zeros(NW, np.int64)
    s = plan["s"]
    for ch in chunks:
        for j in range(ch["nw"]):
            w = ch["w0"] + j
            wslot[w] = ch["slot_base"] + (s[w] - s[ch["w0"]])

    # per-slot tile-first-window bias: radj = row - (base window)*128 where
    # base window = window owning the first slot of the slot's tile.
    # For a window starting mid-tile, its first slots get +128 bias.
    idx16 = np.zeros((NCORE, TS), np.int16)
    radj = np.full((NCORE, TS), -1000.0, np.float32)
    vev = np.zeros((NCORE, TS), np.float32)
    vod = np.zeros((NCORE, TS), np.float32)

    for k in range(NCORE):
        for i in range(NW):
            g = k * NW + i
            a, b = int(starts[g]), int(starts[g + 1])
            n = b - a
            if n == 0:
                continue
            p0 = int(wslot[i])                      # first slot of window i
            sl = np.arange(p0, p0 + n)
            c = cols[a:b]
            idx16[k, sl] = (c >> 1).astype(np.int16)
            # variant bias: slots whose tile starts before the window start
            tile_of = sl // 128
            first_tile_start = tile_of * 128
            bias = np.where(first_tile_start < p0, 128, 0)
            # only the window's FIRST tile can start before p0
            radj[k, sl] = (rows[a:b] - g * W).astype(np.float32) + bias
            par = (c & 1).astype(np.float32)
            v = vals[a:b]
            vev[k, sl] = v * (1.0 - par)
            vod[k, sl] = v * par

    def wrap16(a):
        t = a.reshape(-1, 16).T
        return np.ascontiguousarray(np.tile(t, (8, 1)))

    seqb = np.asarray(seq).reshape(N_NODES, D_FEAT).astype(ml_dtypes.bfloat16)
    tab = np.ascontiguousarray(seqb.reshape(NTOK, 128))

    bf = ml_dtypes.bfloat16
    in_maps = []
    for k in range(NCORE):
        in_maps.append(
            {
                "tab": tab,
                "idx": wrap16(idx16[k]),
                "radj": np.ascontiguousarray(radj[k].reshape(T, 128).T).astype(bf),
                "vev": np.ascontiguousarray(vev[k].reshape(T, 128).T).astype(bf),
                "vod": np.ascontiguousarray(vod[k].reshape(T, 128).T).astype(bf),
            }
        )
    return plan, in_maps


def kernel(seq, vals, rows, cols, _trace=False):
    from concourse.bass_utils import run_bass_kernel_spmd

    plan, in_maps = _preprocess(seq, vals, rows, cols)

    key = tuple(int(x) for x in plan["maxcnt"])
    if key not in _compiled:
        _compiled[key] = _build_kernel(plan)
    nc = _compiled[key]

    res = run_bass_kernel_spmd(nc, in_maps, core_ids=list(range(NCORE)), trace=_trace)

    outs = []
    for k in range(NCORE):
        o = res.results[k]["out"]                        # [128, 3136]
        outs.append(o.reshape(128, NW, 64).transpose(1, 0, 2).reshape(NW * W, 64))
    full = np.concatenate(outs, axis=0)[:N_NODES]
    out = full[None].astype(np.float32)
    if _trace:
        return out, res
    return out


# revision 25
# speedup vs baseline: 1.8865x; 1.0602x over previous
"""Edge-parallel COO SpMM on 8 trn2 cores — 4-queue SWDGE gather variant.

out[r] = sum_{e: rows[e]==r} vals[e] * seq[0, cols[e], :]

rows sorted -> core k owns 49 contiguous 128-row windows (6272 rows).
Per core:
  - tab[t] = [bf16(seq[2t]) | bf16(seq[2t+1])]  (256B tokens, 25000 rows);
    gather idx = col>>1 fits int16.
  - slots laid out window-major, each window padded only to the cross-core
    max count (NOT 128-aligned); chunk = 6-7 windows, padded to 128-multiple.
  - TWO dma_gather halves per chunk, round-robin over the 4 SWDGE queues.
    Q7 desc-gen (~8ns/idx) runs on a different Q7 cpu pair per queue and
    overlaps across queues (~3x measured) — this is the main speedup vs the
    single-queue variant.
  - DVE: one is_equal onehot S0 per half (vs iota 0..127); tiny S1 onehots
    (vs iota 128..255) for tiles where a window starts mid-tile; G scaled
    in-place by vev/vod (vals folded into G, not into two onehot copies).
  - PE per (window, tile): psum_w += S^T @ G_even + S^T @ G_odd.
  - flush psum -> stage -> one DMA to HBM.
Padding slots: idx=0 (valid token, no trailing-negative tricks), radj=-1000
(no onehot match), vev=vod=0 (zeroes G row) — NaN-safe double zero.
"""

import sys

if "/opt/trn_rl_repo" not in sys.path:
    sys.path.insert(0, "/opt/trn_rl_repo")

import numpy as np
import ml_dtypes

N_NODES = 50000
N_EDGES = 1_250_000
D_FEAT = 64
W = 128           # rows per window
NW = 49           # windows per core
NCORE = 8
NTOK = N_NODES // 2
NQUEUES = 4
CHUNK_SIZES = [4] * 12 + [1]

_compiled = {}


def _make_plan(maxcnt):
    """Compile-time schedule from per-window cross-core max counts."""
    maxcnt = np.maximum(np.asarray(maxcnt, np.int64), 128)
    s = np.concatenate([[0], np.cumsum(maxcnt)])      # [NW+1] window offsets
    chunks = []
    tile_base = 0          # global tile index of chunk start
    w0 = 0
    for nw in CHUNK_SIZES:
        o = s[w0:w0 + nw + 1] - s[w0]                 # in-chunk window offsets
        nslots = int(o[-1])
        Cc = -(-nslots // 128)
        # per-window tile spans + variant (0: window starts at/before tile
        # start -> compare vs iota[0:128]; 1: window starts mid-tile ->
        # its radj is +128 biased -> compare vs iota[128:256])
        wplans = []
        btiles = set()
        for j in range(nw):
            t0 = int(o[j]) // 128
            t1 = -(-int(o[j + 1]) // 128)
            tiles = []
            for c in range(t0, t1):
                var = 1 if int(o[j]) > 128 * c else 0
                if var:
                    btiles.add(c)
                tiles.append((c, var, c == t0, c == t1 - 1))
            wplans.append(tiles)
        btiles = sorted(btiles)
        bpos = {c: i for i, c in enumerate(btiles)}
        Ch = (Cc + 1) // 2
        chunks.append(dict(
            w0=w0, nw=nw, Cc=Cc, Ch=Ch, tile_base=tile_base,
            wplans=wplans, btiles=btiles, bpos=bpos,
            slot_base=None,  # filled below
        ))
        tile_base += Cc
        w0 += nw
    T_TILES = tile_base
    # slot base of each chunk in the packed per-core arrays
    sb = 0
    for ch in chunks:
        ch["slot_base"] = sb
        sb += ch["Cc"] * 128
    return dict(maxcnt=maxcnt, s=s, chunks=chunks, T_TILES=T_TILES)


def _build_kernel(plan):
    from concourse import bass, bacc, mybir
    import concourse.tile as tile

    f32 = mybir.dt.float32
    bf16 = mybir.dt.bfloat16
    i16 = mybir.dt.int16

    T = plan["T_TILES"]

    nc = bacc.Bacc("TRN2", target_bir_lowering=False, debug=False,
                   num_swdge_queues=NQUEUES)
    tab_t = nc.dram_tensor("tab", [NTOK, 128], bf16, kind="ExternalInput")
    idx_t = nc.dram_tensor("idx", [128, T * 8], i16, kind="ExternalInput")
    radj_t = nc.dram_tensor("radj", [128, T], bf16, kind="ExternalInput")
    vev_t = nc.dram_tensor("vev", [128, T], bf16, kind="ExternalInput")
    vod_t = nc.dram_tensor("vod", [128, T], bf16, kind="ExternalInput")
    out_t = nc.dram_tensor("out", [128, NW * 64], f32, kind="ExternalOutput")

    # round-robin queue assignment
    qctr = [0]

    def pick_queue(n):
        q = qctr[0] % NQUEUES
        qctr[0] += 1
        return q

    with tile.TileContext(nc) as tc:
        with (
            tc.tile_pool(name="const", bufs=1) as constp,
            tc.tile_pool(name="g", bufs=4) as gp,
            tc.tile_pool(name="s", bufs=1) as sp,
            tc.tile_pool(name="ps", bufs=4, space="PSUM") as psp,
            tc.tile_pool(name="st", bufs=1) as stp,
        ):
            iota_t = constp.tile([128, 256], bf16, name="iota")
            nc.gpsimd.iota(
                iota_t[:],
                pattern=[[1, 256]],
                base=0,
                channel_multiplier=0,
                allow_small_or_imprecise_dtypes=True,
            )
            # all metadata upfront: no per-chunk meta dependency chains
            idxa = constp.tile([128, T * 8], i16, name="idxa")
            rada = constp.tile([128, T], bf16, name="rada")
            veva = constp.tile([128, T], bf16, name="veva")
            voda = constp.tile([128, T], bf16, name="voda")
            nc.sync.dma_start(out=idxa[:], in_=idx_t[:, :])
            nc.sync.dma_start(out=rada[:], in_=radj_t[:, :])
            nc.sync.dma_start(out=veva[:], in_=vev_t[:, :])
            nc.sync.dma_start(out=voda[:], in_=vod_t[:, :])
            stage = stp.tile([128, NW * 64], f32, name="stage")

            for ch in plan["chunks"]:
                Cc, Ch = ch["Cc"], ch["Ch"]
                tb = ch["tile_base"]
                w0, nw = ch["w0"], ch["nw"]

                G = gp.tile([128, Cc, 128], bf16, tag="g")
                for h0, h1 in ((0, Ch), (Ch, Cc)):
                    nc.gpsimd.dma_gather(
                        G[:, h0:h1, :], tab_t[:, :],
                        idxa[:, (tb + h0) * 8:(tb + h1) * 8],
                        (h1 - h0) * 128, (h1 - h0) * 128, 128,
                        single_packet=False,
                        queue_num=pick_queue(h1 - h0),
                    )

                ps = psp.tile([128, 512], f32, tag="ps")
                S1l = {}
                for h0, h1 in ((0, Ch), (Ch, Cc)):
                    hc = h1 - h0
                    S0 = sp.tile([128, hc, 128], bf16, tag=f"s0{h0 > 0}")
                    nc.vector.tensor_tensor(
                        out=S0[:, :, :],
                        in0=iota_t[:, None, 0:128].to_broadcast([128, hc, 128]),
                        in1=rada[:, tb + h0:tb + h1].to_broadcast([128, hc, 128]),
                        op=mybir.AluOpType.is_equal,
                    )
                    bt_h = [c for c in ch["btiles"] if h0 <= c < h1]
                    if bt_h:
                        S1 = sp.tile([128, len(bt_h), 128], bf16,
                                     tag=f"s1{h0 > 0}")
                        S1l[h0 > 0] = (S1, {c: i for i, c in enumerate(bt_h)})
                        for i, c in enumerate(bt_h):
                            nc.vector.tensor_tensor(
                                out=S1[:, i:i + 1, :],
                                in0=iota_t[:, None, 128:256].to_broadcast(
                                    [128, 1, 128]),
                                in1=rada[:, tb + c:tb + c + 1].to_broadcast(
                                    [128, 1, 128]),
                                op=mybir.AluOpType.is_equal,
                            )
                    # fold vals into G (even nodes *= vev, odd *= vod)
                    nc.vector.tensor_tensor(
                        out=G[:, h0:h1, 0:64],
                        in0=G[:, h0:h1, 0:64],
                        in1=veva[:, tb + h0:tb + h1].to_broadcast([128, hc, 64]),
                        op=mybir.AluOpType.mult,
                    )
                    nc.vector.tensor_tensor(
                        out=G[:, h0:h1, 64:128],
                        in0=G[:, h0:h1, 64:128],
                        in1=voda[:, tb + h0:tb + h1].to_broadcast([128, hc, 64]),
                        op=mybir.AluOpType.mult,
                    )
                    for j in range(nw):
                        for (c, var, first, last) in ch["wplans"][j]:
                            if not (h0 <= c < h1):
                                continue
                            if var:
                                S1, pmap = S1l[h0 > 0]
                                lhsT = S1[:, pmap[c], :]
                            else:
                                lhsT = S0[:, c - h0, :]
                            nc.tensor.matmul(
                                out=ps[:, j * 64:(j + 1) * 64],
                                lhsT=lhsT,
                                rhs=G[:, c, 0:64],
                                start=first,
                                stop=False,
                            )
                            nc.tensor.matmul(
                                out=ps[:, j * 64:(j + 1) * 64],
                                lhsT=lhsT,
                                rhs=G[:, c, 64:128],
                                start=False,
                                stop=last,
                            )

                nc.scalar.activation(
                    out=stage[:, w0 * 64:(w0 + nw) * 64],
                    in_=ps[:, :nw * 64],
                    func=mybir.ActivationFunctionType.Copy,
                )
                nc.sync.dma_start(
                    out=out_t[:, w0 * 64:(w0 + nw) * 64],
                    in_=stage[:, w0 * 64:(w0 + nw) * 64],
                )

    nc.compile()
    return nc


def _preprocess(seq, vals, rows, cols):
    rows = np.asarray(rows)
    cols = np.asarray(cols)
    vals = np.asarray(vals)

    w_of_edge = rows // W
    counts = np.bincount(w_of_edge, minlength=NCORE * NW)
    starts = np.concatenate([[0], np.cumsum(counts)])
    cnt = counts.reshape(NCORE, NW)
    maxcnt = np.maximum(cnt.max(axis=0), 128)

    plan = _make_plan(maxcnt)
    chunks = plan["chunks"]
    T = plan["T_TILES"]
    TS = T * 128

    # window -> (chunk, in-chunk offset) slot position
    wslot = np.zeros(NW, np.int64)
    s = plan["s"]
    for ch in chunks:
        for j in range(ch["nw"]):
            w = ch["w0"] + j
            wslot[w] = ch["slot_base"] + (s[w] - s[ch["w0"]])

    # per-slot tile-first-window bias: radj = row - (base window)*128 where
    # base window = window owning the first slot of the slot's tile.
    # For a window starting mid-tile, its first slots get +128 bias.
    idx16 = np.zeros((NCORE, TS), np.int16)
    radj = np.full((NCORE, TS), -1000.0, np.float32)
    vev = np.zeros((NCORE, TS), np.float32)
    vod = np.zeros((NCORE, TS), np.float32)

    for k in range(NCORE):
        for i in range(NW):
            g = k * NW + i
            a, b = int(starts[g]), int(starts[g + 1])
            n = b - a
            if n == 0:
                continue
            p0 = int(wslot[i])                      # first slot of window i
            sl = np.arange(p0, p0 + n)
            c = cols[a:b]
            idx16[k, sl] = (c >> 1).astype(np.int16)
            # variant bias: slots whose tile starts before the window start
            tile_of = sl // 128
            first_tile_start = tile_of * 128
            bias = np.where(first_tile_start < p0, 128, 0)
            # only the window's FIRST tile can start before p0
            radj[k, sl] = (rows[a:b] - g * W).astype(np.float32) + bias
            par = (c & 1).astype(np.float32)
            v = vals[a:b]
            vev[k, sl] = v * (1.0 - par)
            vod[k, sl] = v * par

    def wrap16(a):
        t = a.reshape(-1, 16).T
        return np.ascontiguousarray(np.tile(t, (8, 1)))

    seqb = np.asarray(seq).reshape(N_NODES, D_FEAT).astype(ml_dtypes.bfloat16)
    tab = np.ascontiguousarray(seqb.reshape(NTOK, 128))

    bf = ml_dtypes.bfloat16
    in_maps = []
    for k in range(NCORE):
        in_maps.append(
            {
                "tab": tab,
                "idx": wrap16(idx16[k]),
                "radj": np.ascontiguousarray(radj[k].reshape(T, 128).T).astype(bf),
                "vev": np.ascontiguousarray(vev[k].reshape(T, 128).T).astype(bf),
                "vod": np.ascontiguousarray(vod[k].reshape(T, 128).T).astype(bf),
            }
        )
    return plan, in_maps


def kernel(seq, vals, rows, cols, _trace=False):
    from concourse.bass_utils import run_bass_kernel_spmd

    plan, in_maps = _preprocess(seq, vals, rows, cols)

    key = tuple(int(x) for x in plan["maxcnt"])
    if key not in _compiled:
        _compiled[key] = _build_kernel(plan)
    nc = _compiled[key]

    res = run_bass_kernel_spmd(nc, in_maps, core_ids=list(range(NCORE)), trace=_trace)

    outs = []
    for k in range(NCORE):
        o = res.results[k]["out"]                        # [128, 3136]
        outs.append(o.reshape(128, NW, 64).transpose(1, 0, 2).reshape(NW * W, 64))
    full = np.concatenate(outs, axis=0)[:N_NODES]
    out = full[None].astype(np.float32)
    if _trace:
        return out, res
    return out
